# revision 1
# baseline (speedup 1.0000x reference)
"""Trainium2 Bass kernel for the BDH block (LN + neuron gating + causal RoPE
attention + permuted-reshape z @ encoder + residual + LN).

Sharding (8 NeuronCores): 2-way data parallel on batch x 4-way tensor
parallel. Within each 4-core group, attention is sharded by head pairs
(2 heads/core, all-reduce on attn_out), and the neuron/encoder stages are
sharded on a 1024-wide slice of each head's 4096 neurons (all-reduce after
z @ encoder). All matmuls run as float32r (TF32-like, full PE rate).
"""

import numpy as np

B, T, D, H = 2, 1024, 512, 8
N = 32768
HD = D // H          # 64
NH = N // H          # 4096
EPS = 1e-5

N_CORES = 8
TPG = 4              # cores per data-parallel group
HPC = 2              # heads per core (attention sharding)
NSL = NH // TPG      # 1024: per-core slice of each head's neurons
KT = D // 128        # 4 k-tiles over D
TT = T // 128        # 8 t-tiles
V = H                # 8: t-residue factor in the permuting reshape
UP = T // V          # 128: u per head-row-block

_RUNNER = None


def _host_shards(inputs):
    """Build the 8 per-core input maps from the full inputs."""
    x = np.asarray(inputs["x"], dtype=np.float32)
    Wq = np.asarray(inputs["Wq"], dtype=np.float32)
    Wk = np.asarray(inputs["Wk"], dtype=np.float32)
    Wv = np.asarray(inputs["Wv"], dtype=np.float32)
    Wo = np.asarray(inputs["Wo"], dtype=np.float32)
    dx = np.asarray(inputs["decoder_x"], dtype=np.float32)
    dy = np.asarray(inputs["decoder_y"], dtype=np.float32)
    enc = np.asarray(inputs["encoder"], dtype=np.float32)

    # rope tables in deinterleaved-row layout [128, T] (2 heads stacked; both
    # head slots share the same frequency table)
    inv_freq = 1.0 / (10000.0 ** (np.arange(0, HD, 2, dtype=np.float32) / HD))
    freqs = np.arange(T, dtype=np.float32)[:, None] * inv_freq[None, :]  # [T, 32]
    cos_t = np.cos(freqs).T  # [32, T]
    sin_t = np.sin(freqs).T
    c64 = np.concatenate([cos_t, cos_t], axis=0)            # [64, T]
    s64 = np.concatenate([-sin_t, sin_t], axis=0)           # [64, T]
    cos2 = np.concatenate([c64, c64], axis=0).astype(np.float32)  # [128, T]
    sin2 = np.concatenate([s64, s64], axis=0).astype(np.float32)

    # column permutations for q/k weight slices (deinterleave + swap)
    deint = np.concatenate([np.arange(0, HD, 2), np.arange(1, HD, 2)])  # [64]
    swap = np.concatenate([np.arange(32, 64), np.arange(0, 32)])        # [64]
    perm = np.concatenate([deint, HD + deint])                           # [128]
    perm_s = np.concatenate([deint[swap], HD + deint[swap]])

    # causal masks for the transposed-scores diagonal band, [4, 128, 512]
    masks = np.zeros((4, 128, 512), dtype=np.float32)
    ii = np.arange(128)[:, None]
    jj = np.arange(512)[None, :]
    for c in range(4):
        q = jj // 128
        jloc = jj % 128
        masks[c] = np.where(q < c, 0.0, np.where(q == c, (ii <= jloc).astype(np.float32), 1.0))

    ident = np.eye(128, dtype=np.float32)
    ones1 = np.ones((128, 1), dtype=np.float32)

    enc_r = enc.reshape(V, NH, D)

    in_maps = []
    for c in range(N_CORES):
        b = c // TPG
        r = c % TPG
        rows = slice(r * 128, (r + 1) * 128)   # q/k/v weight rows (2 heads)
        wqT = Wq[rows, :].T.copy()             # [512, 128]
        wkT = Wk[rows, :].T.copy()
        wvT = Wv[rows, :].T.copy()
        woT = Wo[:, rows].T.copy()             # [128, 512]
        dx_c = dx[:, :, r * NSL:(r + 1) * NSL].transpose(1, 0, 2).reshape(D, H * NSL)
        dy_c = dy[:, :, r * NSL:(r + 1) * NSL].transpose(1, 0, 2).reshape(D, H * NSL)
        # re-layout so each 128-column tile is one contiguous [128, 512] DMA:
        # dx2[nt, p, k*128+c] = dx_c[k*128+p, nt*128+c]
        dx_c = dx_c.reshape(KT, 128, H * NSL // 128, 128).transpose(2, 1, 0, 3).reshape(H * NSL // 128, 128, D)
        dy_c = dy_c.reshape(KT, 128, H * NSL // 128, 128).transpose(2, 1, 0, 3).reshape(H * NSL // 128, 128, D)
        enc_c = enc_r[:, r * NSL:(r + 1) * NSL, :].reshape(V * NSL, D)
        in_maps.append({
            "x_in": np.ascontiguousarray(x[b]),
            "wqT": np.ascontiguousarray(wqT[:, perm]),
            "wqTs": np.ascontiguousarray(wqT[:, perm_s]),
            "wkT": np.ascontiguousarray(wkT[:, perm]),
            "wkTs": np.ascontiguousarray(wkT[:, perm_s]),
            "wvT": np.ascontiguousarray(wvT),
            "woT": np.ascontiguousarray(woT),
            "dx_in": np.ascontiguousarray(dx_c),
            "dy_in": np.ascontiguousarray(dy_c),
            "enc_in": np.ascontiguousarray(enc_c),
            "cos2": cos2,
            "sin2": sin2,
            "masks_in": masks,
            "ident": ident,
            "ones1": ones1,
        })
    return in_maps


def _build_program(collectives=True, n_devices=None, repeat_d=1):
    import concourse.bacc as bacc
    import concourse.tile as tile
    from concourse import mybir

    f32 = mybir.dt.float32
    f32r = mybir.dt.float32r
    AF = mybir.ActivationFunctionType

    if n_devices is None:
        n_devices = N_CORES if collectives else 1
    nc = bacc.Bacc("TRN2", target_bir_lowering=False, debug=False,
                   num_devices=n_devices)

    def all_reduce(ins_ap, outs_ap):
        if collectives:
            nc.gpsimd.collective_compute(
                "AllReduce", mybir.AluOpType.add,
                replica_groups=[[0, 1, 2, 3], [4, 5, 6, 7]],
                ins=[ins_ap], outs=[outs_ap])
        else:
            nc.sync.dma_start(out=outs_ap, in_=ins_ap)

    def din(name, shape, dt=f32r):
        return nc.dram_tensor(name, shape, dt, kind="ExternalInput").ap()

    x_in = din("x_in", [T, D], f32)
    wqT = din("wqT", [D, 128]); wqTs = din("wqTs", [D, 128])
    wkT = din("wkT", [D, 128]); wkTs = din("wkTs", [D, 128])
    wvT = din("wvT", [D, 128])
    woT = din("woT", [128, D])
    dx_in = din("dx_in", [H * NSL // 128, 128, D])
    dy_in = din("dy_in", [H * NSL // 128, 128, D])
    enc_in = din("enc_in", [V * NSL, D])
    cos2 = din("cos2", [128, T], f32)
    sin2 = din("sin2", [128, T], f32)
    masks_in = din("masks_in", [4, 128, 512])
    ident = din("ident", [128, 128], f32)
    ones1 = din("ones1", [128, 1])

    y_out = nc.dram_tensor("y_out", [T, D], f32, kind="ExternalOutput").ap()


    with tile.TileContext(nc) as tc:
        with tc.tile_pool(name="const", bufs=1) as const, \
             tc.tile_pool(name="persist", bufs=1) as persist, \
             tc.tile_pool(name="dram", bufs=1, space="DRAM") as dram, \
             tc.tile_pool(name="stats", bufs=4) as stats:

            eps_t = const.tile([128, 1], f32)
            nc.vector.memset(eps_t[:], EPS)

            def ln_tile(out_ap, in_ap):
                st = stats.tile([128, 6], f32, tag="ln_st")
                nc.vector.bn_stats(out=st[:], in_=in_ap)
                mv = stats.tile([128, 2], f32, tag="ln_mv")
                nc.vector.bn_aggr(out=mv[:], in_=st[:])
                sd = stats.tile([128, 1], f32, tag="ln_sd")
                nc.scalar.activation(out=sd[:], in_=mv[:, 1:2], func=AF.Sqrt,
                                     bias=eps_t[:])
                rs = stats.tile([128, 1], f32, tag="ln_rs")
                nc.vector.reciprocal(out=rs[:], in_=sd[:])
                nc.vector.tensor_scalar(out=out_ap, in0=in_ap,
                                        scalar1=mv[:, 0:1], scalar2=rs[:],
                                        op0=mybir.AluOpType.subtract,
                                        op1=mybir.AluOpType.mult)

            # persistent SBUF tensors
            x_sb = persist.tile([128, TT, D], f32)       # x, natural [t,d]
            xnT = persist.tile([128, KT, T], f32r)       # LN(x) transposed
            lnT = persist.tile([128, KT, T], f32r)       # LN(attn) transposed
            ident_sb = const.tile([128, 128], f32)
            nc.sync.dma_start(out=ident_sb[:], in_=ident[:])

            # ---------------- Stage A: load x, LN, transpose ----------------
            with tc.tile_pool(name="stA", bufs=2) as stA, \
                 tc.tile_pool(name="psA", bufs=2, space="PSUM") as psA:
                xn_sb = persist.tile([128, TT, D], f32)
                for i in range(TT):
                    nc.sync.dma_start(out=x_sb[:, i, :], in_=x_in[i * 128:(i + 1) * 128, :])
                    ln_tile(xn_sb[:, i, :], x_sb[:, i, :])
                for i in range(TT):
                    for k in range(KT):
                        ps_tr = psA.tile([128, 128], f32, tag="tr")
                        nc.tensor.transpose(ps_tr[:], xn_sb[:, i, k * 128:(k + 1) * 128], ident_sb[:])
                        nc.vector.tensor_copy(out=xnT[:, k, i * 128:(i + 1) * 128], in_=ps_tr[:])

            # ---------------- Stage B: attention (2 local heads) -------------
            # B1: q/k (roped, transposed) and v (natural)
            qrot = persist.tile([128, T], f32r)
            krot = persist.tile([128, T], f32r)
            v_sb = persist.tile([128, TT, 128], f32r)
            with tc.tile_pool(name="stB1", bufs=2) as stB1, \
                 tc.tile_pool(name="wB1", bufs=1) as wB1, \
                 tc.tile_pool(name="psB1", bufs=2, space="PSUM") as psB1, \
                 tc.tile_pool(name="psV", bufs=2, space="PSUM") as psV:
                w_q = wB1.tile([128, KT, 128], f32r, tag="wq")
                w_qs = wB1.tile([128, KT, 128], f32r, tag="wqs")
                w_k = wB1.tile([128, KT, 128], f32r, tag="wk")
                w_ks = wB1.tile([128, KT, 128], f32r, tag="wks")
                w_v = wB1.tile([128, KT, 128], f32r, tag="wv")
                for (w_t, w_d) in ((w_q, wqT), (w_qs, wqTs), (w_k, wkT), (w_ks, wkTs), (w_v, wvT)):
                    nc.sync.dma_start(out=w_t[:], in_=w_d.rearrange("(k p) j -> p k j", p=128))
                cos_sb = const.tile([128, T], f32)
                sin_sb = const.tile([128, T], f32)
                nc.sync.dma_start(out=cos_sb[:], in_=cos2[:])
                nc.sync.dma_start(out=sin_sb[:], in_=sin2[:])

                for f in range(2):
                    tsl = slice(f * 512, (f + 1) * 512)
                    for (wa, wb, rot) in ((w_q, w_qs, qrot), (w_k, w_ks, krot)):
                        ps_a = psB1.tile([128, 512], f32, tag="ps_a")
                        ps_b = psB1.tile([128, 512], f32, tag="ps_b")
                        for k in range(KT):
                            nc.tensor.matmul(ps_a[:], wa[:, k, :], xnT[:, k, tsl],
                                             start=(k == 0), stop=(k == KT - 1))
                        for k in range(KT):
                            nc.tensor.matmul(ps_b[:], wb[:, k, :], xnT[:, k, tsl],
                                             start=(k == 0), stop=(k == KT - 1))
                        t1 = stB1.tile([128, 512], f32, tag="ropetmp1")
                        t2 = stB1.tile([128, 512], f32, tag="ropetmp2")
                        nc.vector.tensor_mul(out=t1[:], in0=ps_a[:], in1=cos_sb[:, tsl])
                        nc.vector.tensor_mul(out=t2[:], in0=ps_b[:], in1=sin_sb[:, tsl])
                        nc.vector.tensor_add(out=rot[:, tsl], in0=t1[:], in1=t2[:])
                # v natural [t, j]
                for i in range(TT):
                    ps_v = psV.tile([128, 128], f32, tag="ps_v")
                    for k in range(KT):
                        nc.tensor.matmul(ps_v[:], xnT[:, k, i * 128:(i + 1) * 128], w_v[:, k, :],
                                         start=(k == 0), stop=(k == KT - 1))
                    nc.vector.tensor_copy(out=v_sb[:, i, :], in_=ps_v[:])

            # B2: scores, exp, denominators, attn @ v, Wo projection
            ap_dram = dram.tile([T, D], f32, tag="ar1_in")
            ar1_out = dram.tile([T, D], f32, tag="ar1_out")
            den_dram = dram.tile([2, T], f32, tag="den")
            with tc.tile_pool(name="stB2", bufs=4) as stB2, \
                 tc.tile_pool(name="stB2b", bufs=2) as stB2b, \
                 tc.tile_pool(name="wB2", bufs=1) as wB2, \
                 tc.tile_pool(name="psS", bufs=2, space="PSUM") as psS, \
                 tc.tile_pool(name="psDen", bufs=1, space="PSUM") as psDen, \
                 tc.tile_pool(name="psAv", bufs=1, space="PSUM") as psAv, \
                 tc.tile_pool(name="psAp", bufs=2, space="PSUM") as psAp:
                masks_sb = wB2.tile([128, 4, 512], f32r, tag="masks")
                nc.sync.dma_start(out=masks_sb[:], in_=masks_in.rearrange("c p n -> p c n"))
                ones_sb = wB2.tile([128, 1], f32r, tag="ones")
                nc.sync.dma_start(out=ones_sb[:], in_=ones1[:])
                wo_h = [wB2.tile([64, D], f32r, tag=f"wo{h}", name=f"wo_h{h}")
                        for h in range(2)]
                for h in range(2):
                    nc.sync.dma_start(out=wo_h[h][:], in_=woT[h * 64:(h + 1) * 64, :])

                avn = [persist.tile([64, T], f32r, name=f"avn{h}") for h in range(2)]
                av_raw = [persist.tile([64, T], f32, name=f"av_raw{h}") for h in range(2)]
                den_bc = [persist.tile([64, T], f32, name=f"den_bc{h}") for h in range(2)]

                for f in range(2):
                    tsl = slice(f * 512, (f + 1) * 512)
                    av_ps = [psAv.tile([64, 512], f32, tag=f"av{h}", name=f"av_ps{h}")
                             for h in range(2)]
                    np_tiles = 4 * f + 4
                    den_ps = [psDen.tile([1, 512], f32, tag=f"den{h}", name=f"den_ps{h}")
                              for h in range(2)]
                    for p in range(np_tiles):
                        for h in range(2):
                            hsl = slice(h * 64, (h + 1) * 64)
                            s_ps = psS.tile([128, 512], f32, tag="s")
                            nc.tensor.matmul(s_ps[:], krot[hsl, p * 128:(p + 1) * 128],
                                             qrot[hsl, tsl], start=True, stop=True)
                            e_sb = stB2.tile([128, 512], f32r, tag="exp")
                            nc.scalar.activation(out=e_sb[:], in_=s_ps[:], func=AF.Exp)
                            cstar = p - 4 * f
                            if cstar >= 0:
                                nc.vector.tensor_mul(out=e_sb[:], in0=e_sb[:],
                                                     in1=masks_sb[:, cstar, :])
                            nc.tensor.matmul(den_ps[h][:], ones_sb[:], e_sb[:],
                                             start=(p == 0), stop=(p == np_tiles - 1))
                            nc.tensor.matmul(av_ps[h][:], v_sb[:, p, hsl], e_sb[:],
                                             start=(p == 0), stop=(p == np_tiles - 1))
                    for h in range(2):
                        nc.vector.tensor_copy(out=av_raw[h][:, tsl], in_=av_ps[h][:])
                        dr = stB2b.tile([1, 512], f32, tag="denrow")
                        nc.vector.reciprocal(out=dr[:], in_=den_ps[h][:])
                        nc.sync.dma_start(out=den_dram[h:h + 1, tsl], in_=dr[:])
                for h in range(2):
                    nc.gpsimd.dma_start(out=den_bc[h][:],
                                        in_=den_dram[h:h + 1, :].partition_broadcast(64))
                    nc.vector.tensor_mul(out=avn[h][:], in0=av_raw[h][:], in1=den_bc[h][:])
                for i in range(TT):
                    ap_ps = psAp.tile([128, 512], f32, tag="ap")
                    for h in range(2):
                        nc.tensor.matmul(ap_ps[:], avn[h][:, i * 128:(i + 1) * 128],
                                         wo_h[h][:], start=(h == 0), stop=(h == 1))
                    o_sb = stB2b.tile([128, 512], f32, tag="apout")
                    nc.vector.tensor_copy(out=o_sb[:], in_=ap_ps[:])
                    nc.sync.dma_start(out=ap_dram[i * 128:(i + 1) * 128, :], in_=o_sb[:])

            all_reduce(ap_dram.opt(), ar1_out.opt())

            # ---------------- Stage C: LN(attn_out), transpose ----------------
            with tc.tile_pool(name="stC", bufs=2) as stC, \
                 tc.tile_pool(name="psC", bufs=2, space="PSUM") as psC:
                af_sb = stC.tile([128, TT, D], f32, tag="af")
                for i in range(TT):
                    nc.sync.dma_start(out=af_sb[:, i, :], in_=ar1_out[i * 128:(i + 1) * 128, :])
                    ln_tile(af_sb[:, i, :], af_sb[:, i, :])
                for i in range(TT):
                    for k in range(KT):
                        ps_tr = psC.tile([128, 128], f32, tag="trc")
                        nc.tensor.transpose(ps_tr[:], af_sb[:, i, k * 128:(k + 1) * 128], ident_sb[:])
                        nc.vector.tensor_copy(out=lnT[:, k, i * 128:(i + 1) * 128], in_=ps_tr[:])

            # ---------------- Stage D: neurons, gate, z @ enc ----------------
            ar2_in = dram.tile([T, D], f32, tag="ar2_in")
            ar2_out = dram.tile([T, D], f32, tag="ar2_out")
            out_sb = persist.tile([128, H, D], f32)
            NLB = NSL // 128  # 8 sub-blocks per head slice
            with tc.tile_pool(name="encD", bufs=2) as encD, \
                 tc.tile_pool(name="wD", bufs=3) as wD, \
                 tc.tile_pool(name="actD", bufs=3) as actD, \
                 tc.tile_pool(name="psMM", bufs=6, space="PSUM") as psMM, \
                 tc.tile_pool(name="psZ", bufs=2, space="PSUM") as psZ:
              for _rep in range(repeat_d):
                for nlb in range(NLB):
                    enc_t = encD.tile([128, V, D], f32r, tag="enc")
                    for v in range(V):
                        nc.sync.dma_start(
                            out=enc_t[:, v, :],
                            in_=enc_in[v * NSL + nlb * 128: v * NSL + (nlb + 1) * 128, :])
                    for h in range(H):
                        j = h * NLB + nlb
                        csl = slice(j * 128, (j + 1) * 128)
                        dx_t = wD.tile([128, KT, 128], f32r, tag="dx")
                        dy_t = wD.tile([128, KT, 128], f32r, tag="dy")
                        nc.sync.dma_start(out=dx_t[:], in_=dx_in[j, :, :].rearrange("p (k c) -> p k c", k=KT))
                        nc.sync.dma_start(out=dy_t[:], in_=dy_in[j, :, :].rearrange("p (k c) -> p k c", k=KT))
                        xr = actD.tile([128, T], f32r, tag="xr")
                        yr = actD.tile([128, T], f32r, tag="yr")
                        for (w_t, src, dst) in ((dx_t, xnT, xr), (dy_t, lnT, yr)):
                            for tb in range(2):
                                tsl = slice(tb * 512, (tb + 1) * 512)
                                mm_ps = psMM.tile([128, 512], f32, tag="mm")
                                for k in range(KT):
                                    nc.tensor.matmul(mm_ps[:], w_t[:, k, :], src[:, k, tsl],
                                                     start=(k == 0), stop=(k == KT - 1))
                                nc.scalar.activation(out=dst[:, tsl], in_=mm_ps[:], func=AF.Relu)
                        z_sb = actD.tile([128, T], f32r, tag="z")
                        nc.vector.tensor_mul(out=z_sb[:], in0=xr[:], in1=yr[:])
                        zv = z_sb[:].rearrange("p (u v) -> p v u", v=V)
                        z_ps = psZ.tile([128, 512], f32, tag="zacc")
                        for v in range(V):
                            nc.tensor.matmul(z_ps[:], zv[:, v, :], enc_t[:, v, :],
                                             start=(v == 0), stop=(v == V - 1))
                        if nlb == 0:
                            nc.vector.tensor_copy(out=out_sb[:, h, :], in_=z_ps[:])
                        else:
                            nc.vector.tensor_add(out=out_sb[:, h, :], in0=out_sb[:, h, :],
                                                 in1=z_ps[:])
                for h in range(H):
                    nc.sync.dma_start(out=ar2_in[h * 128:(h + 1) * 128, :], in_=out_sb[:, h, :])

            all_reduce(ar2_in.opt(), ar2_out.opt())

            # ---------------- Final: residual + LN ----------------
            with tc.tile_pool(name="stF", bufs=3) as stF:
                for i in range(TT):
                    fo = stF.tile([128, D], f32, tag="fo")
                    nc.sync.dma_start(out=fo[:], in_=ar2_out[i * 128:(i + 1) * 128, :])
                    nc.vector.tensor_add(out=fo[:], in0=fo[:], in1=x_sb[:, i, :])
                    ln_tile(fo[:], fo[:])
                    nc.sync.dma_start(out=y_out[i * 128:(i + 1) * 128, :], in_=fo[:])

    nc.compile()
    return nc


class _Runner:
    """Compile once, jit once, execute many times."""

    def __init__(self):
        import jax
        import numpy as _np
        from jax.sharding import Mesh, PartitionSpec
        from jax.experimental.shard_map import shard_map
        from concourse import bass2jax, mybir

        self.jax = jax
        nc = _build_program()
        self.nc = nc
        bass2jax.install_neuronx_cc_hook()

        in_names, out_names, out_avals, zero_outs = [], [], [], []
        pn = nc.partition_id_tensor.name if nc.partition_id_tensor else None
        for alloc in nc.m.functions[0].allocations:
            if not isinstance(alloc, mybir.MemoryLocationSet):
                continue
            name = alloc.memorylocations[0].name
            if alloc.kind == "ExternalInput":
                if name != pn:
                    in_names.append(name)
            elif alloc.kind == "ExternalOutput":
                out_names.append(name)
                shape = tuple(alloc.tensor_shape)
                dtype = mybir.dt.np(alloc.dtype)
                out_avals.append(jax.core.ShapedArray(shape, dtype))
                zero_outs.append(_np.zeros(shape, dtype))
        self.in_names, self.out_names = in_names, out_names
        self.zero_outs = zero_outs
        n_params = len(in_names)
        all_in = in_names + out_names + ([pn] if pn else [])

        def _body(*args):
            operands = list(args)
            if pn is not None:
                operands.append(bass2jax.partition_id_tensor())
            outs = bass2jax._bass_exec_p.bind(
                *operands, out_avals=tuple(out_avals), in_names=tuple(all_in),
                out_names=tuple(out_names), lowering_input_output_aliases=(),
                sim_require_finite=True, sim_require_nnan=True, nc=nc)
            return tuple(outs)

        devices = jax.devices()[:N_CORES]
        mesh = Mesh(np.asarray(devices), ("core",))
        n_all = n_params + len(out_names)
        self.fn = jax.jit(
            shard_map(_body, mesh=mesh,
                      in_specs=(PartitionSpec("core"),) * n_all,
                      out_specs=(PartitionSpec("core"),) * len(out_names),
                      check_rep=False),
            keep_unused=True)
        self.sharding = jax.sharding.NamedSharding(mesh, PartitionSpec("core"))
        self.out_avals = out_avals

    def device_args(self, in_maps):
        concat_in = [np.concatenate([m[nm] for m in in_maps], axis=0)
                     for nm in self.in_names]
        concat_zero = [np.zeros((N_CORES * z.shape[0], *z.shape[1:]), z.dtype)
                       for z in self.zero_outs]
        return [self.jax.device_put(a, self.sharding)
                for a in concat_in + concat_zero]

    def run(self, dev_args):
        outs = self.fn(*dev_args)
        self.jax.block_until_ready(outs)
        return outs

    def results(self, outs):
        per_core = []
        for c in range(N_CORES):
            per_core.append({
                nm: np.asarray(outs[i]).reshape(N_CORES, *self.out_avals[i].shape)[c]
                for i, nm in enumerate(self.out_names)})
        return per_core


def _get_runner():
    global _RUNNER
    if _RUNNER is None:
        _RUNNER = _Runner()
    return _RUNNER


def kernel(**inputs):
    import time as _time

    in_maps = _host_shards(inputs)
    last_exc = None
    for attempt in range(3):
        try:
            runner = _get_runner()
            outs = runner.run(runner.device_args(in_maps))
            res = runner.results(outs)
            out = np.stack([res[0]["y_out"], res[TPG]["y_out"]], axis=0)
            return out.astype(np.float32)
        except Exception as exc:  # transient device/tunnel hiccups: retry once or twice
            last_exc = exc
            global _RUNNER
            _RUNNER = None
            _time.sleep(3.0)
    raise last_exc

